# revision 31
# baseline (speedup 1.0000x reference)
"""Final kernel (HW: 10134ns, rel err 6.9e-4 vs the 2e-2 harness gate).

Measurement window = [first compute-class instruction -> absolute end of
the execution, including the runtime's fixed ~7.4us semaphore-sweep
teardown that follows every NEFF execution].  A trivial copy kernel
measures ~8.3us on this runtime, so only the compute chain is winnable.

vs the 12.7us baseline:
- TileContext end-block barriers stripped (-1.5us): the blocking out-DMA
  already orders the Sync queue, and the runtime wrapper re-clears every
  semaphore anyway.
- 4 activations instead of 5: ct = sin(pi/2 - t) directly (cos is even,
  so no |t| needed; max arg 2.35 rad is within the Sin PWP range).
- cr/sr written interleaved by the two rot ACTs, so one DVE multiply
  against a host-packed interleaved mag vector forms [mc,ms] pairs, and
  comp0/comp2 are single 3D-broadcast DVE ops (ct/st read stride-0).
- comp1 = [-ms, mc] via two strided DVE ops (Pool is ~4x slower per tiny
  op), scheduled BEFORE the st-gated products so it never gates the PE.
- projection matmuls in fp16 (lhsT = X' packed as fp16 bitcast): 1-pass
  on the PE instead of fp32's 2-pass; issued in product-readiness order
  with the stop flag on the last gate.
- post-passes: drop same-engine sem waits, hoist multi-waits onto
  single-wait carriers (walrus limit), strip framework preamble movs."""

import os

import numpy as np

import concourse.bass as bass
import concourse.mybir as mybir
from concourse import tile
from concourse.tile_rust import add_dep_helper
from concourse.bass_utils import run_bass_kernel_spmd

N_VIEWS = 40
N_MARKERS = 4
N_TILT = 8
PI = float(np.pi)
DEG2RAD = PI / 180.0
HALF_PI = PI / 2.0
CHECK_LIST = np.array([0, 3, 8, 20, 26, 32, 36, 39])
N_CORES = 8

V = N_VIEWS
W2 = 2 * V

# pack layout ([9, 259], row 0 unless noted):
#   col 0              [1; tilt_angles]            (partitions 0..8)
#   cols 1..41         wmat (radians)              (partitions 0..8)
#   col 41             pi/2
#   col 42             0.0
#   cols 43..83        rot_angles (degrees)
#   cols 83..163       mag_eff interleaved [m0,m0,m1,m1,...] (view-0 -> 1)
#   cols 163..243      off_eff flattened (view-0 zeroed)
#   cols 243..259      X'^T rows as [1,4] blocks per seg: [1s, x, y, z]
C_HPI = 1 + V                # 41
C_ZERO = C_HPI + 1           # 42
C_ROT = C_ZERO + 1           # 43
C_MAGI = C_ROT + V           # 83
C_OFF = C_MAGI + W2          # 163
C_XP = C_OFF + W2            # 243: fp32 ones row for the off rank-1
C_XP16 = C_XP + 4            # 247: 8 f32 cells = 16 packed fp16 X' values
C_PM = C_XP16 + 8            # 255: [-1.0, 1.0] pair for the comp1 swap
C_Y4 = C_PM + 2              # 257: xyz[:,1] on partitions 0..3
C_L16 = C_Y4 + 1             # 258: fp16 [ [1;tilt] | wmat ] (9x42 halves)
C_OFF16 = C_L16 + 21         # 279: fp16 off row (80 halves)
C_ONES16 = C_OFF16 + 40      # 319: fp16 ones[4] for the off rank-1
PACK_COLS = C_ONES16 + 2     # 321

KEEP = np.r_[0:40, 58:160]

AFT = mybir.ActivationFunctionType
F32 = mybir.dt.float32
F16 = mybir.dt.float16


def _build_wmat() -> np.ndarray:
    views = np.arange(N_VIEWS)
    idx1 = np.searchsorted(CHECK_LIST, views, side="right") - 1
    idx2 = np.minimum(idx1 + 1, len(CHECK_LIST) - 1)
    denom = (CHECK_LIST[idx2] - CHECK_LIST[idx1]).astype(np.float32)
    denom[denom == 0] = 1.0
    frac = (views - CHECK_LIST[idx1]).astype(np.float32) / denom
    w = np.zeros((N_VIEWS, N_TILT), dtype=np.float64)
    c = np.zeros(N_VIEWS, dtype=np.float64)
    for v in range(N_VIEWS):
        if v == 14:
            c[v] = -15.0
        else:
            w[v, idx1[v]] += 1.0 - float(frac[v])
            w[v, idx2[v]] += float(frac[v])
    wmat = np.concatenate([c[None, :], w.T], axis=0) * DEG2RAD
    return np.ascontiguousarray(wmat, dtype=np.float32)


_WMAT = _build_wmat()
_NC_CACHE: list = []

# Experiment gates (env): BASS_MAX_SEM=<n> adds --max-sem-num to walrus,
# BASS_KEEP_END=1 keeps the TileContext end-block barriers.
_MAX_SEM = os.environ.get("BASS_MAX_SEM")
_STRIP_END = os.environ.get("BASS_KEEP_END") != "1"
_TAG = f"v17_ms{_MAX_SEM or 'x'}se{int(_STRIP_END)}"

_ORIG_WALRUS_ARGS = None


def _install_walrus_patch():
    """Append --max-sem-num to the walrus invocation for our compiles."""
    global _ORIG_WALRUS_ARGS
    if _MAX_SEM is None or _ORIG_WALRUS_ARGS is not None:
        return
    import concourse.bass_utils as bu

    _ORIG_WALRUS_ARGS = bu.get_walrus_args

    def _patched(arch, tmpdir, *, dve_root=None):
        return [f"--max-sem-num={_MAX_SEM}"] + _ORIG_WALRUS_ARGS(
            arch, tmpdir, dve_root=dve_root)

    bu.get_walrus_args = _patched


def _chain(insts):
    for a, b in zip(insts, insts[1:]):
        add_dep_helper(b.ins, a.ins, sync=False, reason="pin engine order")


_ENG_SEM_PREFIX = {
    "Pool": "Pool", "Activation": "Activation", "PE": "PE", "DVE": "DVE",
}

# earliest-firing sems first so carrier stalls overlap earlier waits;
# DVE (the longest product chain) is usually the true gate -> last.
_WAIT_ORDER = {"DMAHW": 0, "Sync": 1, "SP": 1, "Pool": 2, "Activation": 3,
               "PE": 4, "DVE": 5}


def _strip_self_waits(nc) -> None:
    """Drop sem waits on the instruction's own engine: the queue is
    in-order, so they are redundant and would cost a carrier each."""
    for fn in nc.m.functions:
        for blk in fn.blocks:
            for inst in blk.instructions:
                si = inst.sync_info
                if si is None or not si.on_wait:
                    continue
                pref = _ENG_SEM_PREFIX.get(getattr(inst.engine, "value",
                                                   str(inst.engine)))
                if pref is None:
                    continue
                kept = [w for w in si.on_wait
                        if not str(getattr(w, "ant_name", "")).startswith(
                            pref + "_")]
                if len(kept) != len(si.on_wait) and kept:
                    si.on_wait = kept


def _legalize_multiwait(nc) -> None:
    """walrus fits one sem-wait per instruction; hoist extras onto
    single-wait EventSemaphore carriers (earliest-firing first)."""

    def prio(w):
        nm = str(getattr(w, "ant_name", ""))
        base = nm.split("_")[0]
        for k, v in _WAIT_ORDER.items():
            if base.startswith(k):
                return v
        return 9

    for fn in nc.m.functions:
        for blk in fn.blocks:
            il = blk.instructions
            i = 0
            while i < len(il):
                inst = il[i]
                si = inst.sync_info
                if si is not None and si.on_wait is not None and len(si.on_wait) > 1:
                    waits = sorted(si.on_wait, key=prio)
                    extras, keep = waits[:-1], waits[-1]
                    for j, w in enumerate(extras):
                        ev = mybir.InstEventSemaphore(
                            name=f"{inst.name}_wsplit{j}")
                        ev.engine = inst.engine
                        try:
                            ev.sync_info.on_wait = [w]
                        except Exception:
                            ev.sync_info = mybir.SyncInfo(on_wait=[w],
                                                          on_update=[])
                        il.insert(i, ev)
                        i += 1
                    si.on_wait = [keep]
                i += 1


def _strip_preamble(nc) -> None:
    """Drop the const-AP memsets and the init all-engine barrier (nothing
    uses the const APs; all cross-engine ordering is via tile sems)."""
    il = nc.m.functions[0].blocks[0].instructions
    keep = []
    for inst in il:
        nm = type(inst).__name__
        if nm == "InstMemset" and "const-" in str(inst.outs[0]):
            continue
        if nm in ("InstDrain", "InstEventSemaphore", "InstRegisterMove"):
            continue
        keep.append(inst)
    il[:] = keep


def _strip_endblock(nc) -> None:
    """Drop the TileContext exit barriers/sem clears (the Sync queue's
    blocking out-DMA already orders completion; the runtime wrapper
    re-clears every semaphore anyway), then the now-empty end block and
    the per-engine branches into it: Sync's branch ran AFTER the out-DMA
    and extended the measured window by ~120ns."""
    for fn in nc.m.functions:
        for blk in fn.blocks:
            if blk.name.endswith("_end"):
                blk.instructions[:] = [
                    i for i in blk.instructions
                    if type(i).__name__ not in
                    ("InstEventSemaphore", "InstDrain", "InstISA")
                ]
    for fn in nc.m.functions:
        empty_ends = {b.name for b in fn.blocks
                      if b.name.endswith("_end") and not b.instructions}
        if not empty_ends:
            continue
        for blk in fn.blocks:
            blk.instructions[:] = [
                i for i in blk.instructions
                if not (type(i).__name__ == "InstUnconditionalBranch"
                        and str(getattr(i, "target", "")) in empty_ends)
            ]
        fn.blocks[:] = [b for b in fn.blocks if b.name not in empty_ends]


def _build_nc(postpasses: bool = True) -> bass.Bass:
    nc = bass.Bass("TRN2", target_bir_lowering=False, debug=False,
                   num_devices=N_CORES)

    pack_d = nc.dram_tensor(f"pack_{_TAG}", [N_TILT + 1, PACK_COLS], F32,
                            kind="ExternalInput")
    out_d = nc.dram_tensor("out", [4 * V, 2], F32, kind="ExternalOutput")

    with tile.TileContext(nc) as tc:
        with (
            tc.tile_pool(name="sb", bufs=1) as sb,
            tc.tile_pool(name="ps", bufs=1, space="PSUM") as ps,
        ):
            pk = sb.tile([N_TILT + 1, PACK_COLS], F32)
            tilt_ps = ps.tile([1, V], F32)
            tilt_ps2 = ps.tile([1, V], F32)
            crsr = sb.tile([1, W2], F32)   # [cr|sr] interleaved pairs
            ct = sb.tile([1, V], F32)
            st = sb.tile([1, V], F32)
            mcs = sb.tile([1, W2], F32)    # [mc|ms] interleaved pairs
            stage = sb.tile([1, 3 * W2], F16)  # [comp0 | comp1 | comp2]
            uv_ps = ps.tile([N_MARKERS, W2], F32)
            out_sb = sb.tile([N_MARKERS, W2], F32)

            halfpi_ap = pk[0:1, C_HPI:C_HPI + 1]
            zero_ap = pk[0:1, C_ZERO:C_ZERO + 1]
            rot_ap = pk[0:1, C_ROT:C_ROT + V]
            magi_ap = pk[0:1, C_MAGI:C_MAGI + W2]
            off_ap = pk[0:1, C_OFF:C_OFF + W2]

            # ---- single input DMA (free preamble) -----------------------
            d_in = nc.sync.dma_start(pk[:, :], pack_d.ap())

            # ---- PE (all fp16, 1-pass): tilt radians TWICE into two
            # separate PSUM banks (ACT PSUM reads to one bank serialize:
            # a_st waited for a_ct to retire), then the off rank-1 -------
            lw16 = pk[0:9, C_L16:C_L16 + 21].bitcast(F16)   # [9, 42]
            off16 = pk[0:1, C_OFF16:C_OFF16 + 40].bitcast(F16)  # [1, 80]
            ones16 = pk[0:1, C_ONES16:C_ONES16 + 2].bitcast(F16)  # [1, 4]
            mm1 = nc.tensor.matmul(tilt_ps[:, :], lw16[:, 0:1],
                                   lw16[:, 1:1 + V])
            mm1b = nc.tensor.matmul(tilt_ps2[:, :], lw16[:, 0:1],
                                    lw16[:, 1:1 + V])
            mm_off = nc.tensor.matmul(uv_ps[:, :], ones16, off16,
                                      start=True, stop=False)

            # ---- ACT: sr -> cr -> ct -> st ------------------------------
            # sr/cr write interleaved [cr,sr] pairs; ct = sin(pi/2 - t).
            a_sr = nc.scalar.activation(crsr[0:1, 1:W2:2], rot_ap, AFT.Sin,
                                        bias=zero_ap, scale=DEG2RAD)
            a_cr = nc.scalar.activation(crsr[0:1, 0:W2:2], rot_ap, AFT.Sin,
                                        bias=halfpi_ap, scale=DEG2RAD)
            a_ct = nc.scalar.activation(ct[:, :], tilt_ps[:, :], AFT.Sin,
                                        bias=halfpi_ap, scale=-1.0)
            a_st = nc.scalar.activation(st[:, :], tilt_ps2[:, :], AFT.Sin,
                                        bias=zero_ap)
            _chain([a_sr, a_cr, a_ct, a_st])

            # ---- DVE: mcs, then comp0/comp2 as single 3D-broadcast ops --
            s1, s2 = W2, 2 * W2
            mcs3 = mcs[0:1, 0:W2].rearrange("p (v r) -> p v r", v=V, r=2)
            sg0 = stage[0:1, 0:s1].rearrange("p (v r) -> p v r", v=V, r=2)
            sg2 = stage[0:1, s2:3 * W2].rearrange("p (v r) -> p v r", v=V,
                                                  r=2)
            ct_b = ct[0:1, 0:V].unsqueeze(2).broadcast_to([1, V, 2])
            st_b = st[0:1, 0:V].unsqueeze(2).broadcast_to([1, V, 2])
            v1 = nc.vector.tensor_mul(mcs[:, :], magi_ap, crsr[:, :])
            v2 = nc.vector.tensor_mul(sg0, mcs3, ct_b)
            # comp1 = [-ms, mc] from the mcs pairs, scheduled BEFORE the
            # st products: it only needs mcs, and running it last made it
            # gate the stop matmul (DVE is serial; Pool is ~4x slower)
            g1 = nc.vector.tensor_scalar_mul(stage[0:1, s1:s2:2],
                                             mcs[0:1, 1:W2:2], -1.0)
            g2 = nc.vector.tensor_copy(stage[0:1, s1 + 1:s2:2],
                                       mcs[0:1, 0:W2:2])
            v3 = nc.vector.tensor_mul(sg2, mcs3, st_b)
            _chain([v1, v2, g1, g2, v3])

            # ---- PE: accumulate the three component rank-1 terms --------
            xp16 = pk[0:1, C_XP16:C_XP16 + 8].bitcast(F16)
            mm_s0 = nc.tensor.matmul(uv_ps[:, :], xp16[0:1, 0:4],
                                     stage[0:1, 0:s1],
                                     start=False, stop=False)
            mm_s1 = nc.tensor.matmul(uv_ps[:, :], xp16[0:1, 4:8],
                                     stage[0:1, s1:s2],
                                     start=False, stop=False)
            mm_s2 = nc.tensor.matmul(uv_ps[:, :], xp16[0:1, 8:12],
                                     stage[0:1, s2:3 * W2],
                                     start=False, stop=True)
            _chain([mm1, mm1b, mm_off, mm_s0, mm_s1, mm_s2])

            # ---- PSUM -> SBUF -> out DMA --------------------------------
            v_fin = nc.vector.tensor_copy(out_sb[:, :], uv_ps[:, :])
            _chain([v3, v_fin])
            d_out = nc.sync.dma_start(out_d.ap(), out_sb[:, :],
                                      single_packet=True)
            _chain([d_in, d_out])

    if postpasses:
        _strip_self_waits(nc)
        _legalize_multiwait(nc)
        _strip_preamble(nc)
        if _STRIP_END:
            _strip_endblock(nc)
    return nc


def _make_in_map(inputs: dict) -> dict:
    tilt = np.ascontiguousarray(inputs["tilt_angles"], dtype=np.float32)
    xyz = np.ascontiguousarray(inputs["xyz"], dtype=np.float32)
    mag_eff = np.ascontiguousarray(inputs["mag"], np.float32).copy()
    mag_eff[0] = 1.0
    off_eff = np.ascontiguousarray(inputs["offset"], np.float32).copy()
    off_eff[0] = 0.0
    pack = np.zeros((N_TILT + 1, PACK_COLS), np.float32)
    pack[0, 0] = 1.0
    pack[1:, 0] = tilt
    pack[:, 1:1 + N_VIEWS] = _WMAT
    pack[0, C_HPI] = HALF_PI
    pack[0, C_ROT:C_MAGI] = np.ascontiguousarray(inputs["rot_angles"],
                                                 np.float32)
    pack[0, C_MAGI:C_OFF] = np.repeat(mag_eff, 2)
    pack[0, C_OFF:C_XP] = off_eff.reshape(-1)
    # ones row (fp32) for the off rank-1; X' columns packed as fp16
    pack[0, C_XP:C_XP + 4] = 1.0
    h = np.zeros(16, np.float16)
    h[0:4] = xyz[:, 0]
    h[4:8] = xyz[:, 1]
    h[8:12] = xyz[:, 2]
    pack[0, C_XP16:C_XP16 + 8] = h.view(np.float32)
    pack[0, C_PM] = -1.0
    pack[0, C_PM + 1] = 1.0
    pack[0:4, C_Y4] = xyz[:, 1]
    lw = np.zeros((9, 42), np.float16)
    lw[0, 0] = 1.0
    lw[1:, 0] = tilt
    lw[:, 1:41] = _WMAT
    pack[:, C_L16:C_L16 + 21] = lw.view(np.float32)
    o16 = np.zeros(80, np.float16)
    o16[:] = off_eff.reshape(-1)
    pack[0, C_OFF16:C_OFF16 + 40] = o16.view(np.float32)
    e16 = np.ones(4, np.float16)
    pack[0, C_ONES16:C_ONES16 + 2] = e16.view(np.float32)
    return {f"pack_{_TAG}": pack}


def kernel(**inputs: np.ndarray) -> np.ndarray:
    _install_walrus_patch()
    if not _NC_CACHE:
        _NC_CACHE.append(_build_nc())
    nc = _NC_CACHE[0]

    in_map = _make_in_map(inputs)
    core_ids = list(range(N_CORES))
    res = run_bass_kernel_spmd(nc, [in_map] * N_CORES, core_ids)
    full = np.asarray(res.results[0]["out"], dtype=np.float32)
    return np.ascontiguousarray(full[KEEP])


# revision 34
# speedup vs baseline: 1.1909x; 1.1909x over previous
"""Final kernel (HW: ~10.0us at fast clocks, rel err 6.86e-4 vs 2e-2 gate).

Measurement window = [first compute-class instruction -> absolute end of
the execution, including the runtime's fixed ~7.4us semaphore-sweep
teardown that follows every NEFF execution].  A trivial copy kernel
measures ~8.3us on this runtime, so only the compute chain is winnable.

vs the 12.7us baseline:
- TileContext end-block barriers stripped (-1.5us): the blocking out-DMA
  already orders the Sync queue, and the runtime wrapper re-clears every
  semaphore anyway.
- 4 activations instead of 5: ct = sin(pi/2 - t) directly (cos is even,
  so no |t| needed; max arg 2.35 rad is within the Sin PWP range); tilt
  is computed TWICE into two separate PSUM tiles (fp16 1-pass matmuls)
  because ACT PSUM reads to one bank serialize: with a private bank the
  4th activation pipelines at the ~220ns cadence instead of stalling.
- cr/sr written interleaved by the two rot ACTs, so one DVE multiply
  against a host-packed interleaved mag vector forms [mc,ms] pairs, and
  comp0/comp2 are single 3D-broadcast DVE ops (ct/st read stride-0).
- comp1 = [-ms, mc] via two strided DVE ops (Pool is ~4x slower per tiny
  op), scheduled BEFORE the st-gated products so it never gates the PE.
- projection matmuls in fp16 (lhsT = X' packed as fp16 bitcast): 1-pass
  on the PE instead of fp32's 2-pass; issued in product-readiness order
  with the stop flag on the last gate.
- post-passes: drop same-engine sem waits, hoist multi-waits onto
  single-wait carriers (walrus limit), strip framework preamble movs,
  drop the per-engine branches to the empty end block (Sync's ran after
  the out-DMA and extended the measured window)."""

import os

import numpy as np

import concourse.bass as bass
import concourse.mybir as mybir
from concourse import tile
from concourse.tile_rust import add_dep_helper
from concourse.bass_utils import run_bass_kernel_spmd

N_VIEWS = 40
N_MARKERS = 4
N_TILT = 8
PI = float(np.pi)
DEG2RAD = PI / 180.0
HALF_PI = PI / 2.0
CHECK_LIST = np.array([0, 3, 8, 20, 26, 32, 36, 39])
N_CORES = 8

V = N_VIEWS
W2 = 2 * V

# pack layout ([9, 259], row 0 unless noted):
#   col 0              [1; tilt_angles]            (partitions 0..8)
#   cols 1..41         wmat (radians)              (partitions 0..8)
#   col 41             pi/2
#   col 42             0.0
#   cols 43..83        rot_angles (degrees)
#   cols 83..163       mag_eff interleaved [m0,m0,m1,m1,...] (view-0 -> 1)
#   cols 163..243      off_eff flattened (view-0 zeroed)
#   cols 243..259      X'^T rows as [1,4] blocks per seg: [1s, x, y, z]
C_HPI = 1 + V                # 41
C_ZERO = C_HPI + 1           # 42
C_ROT = C_ZERO + 1           # 43
C_MAGI = C_ROT + V           # 83
C_OFF = C_MAGI + W2          # 163
C_XP = C_OFF + W2            # 243: fp32 ones row for the off rank-1
C_XP16 = C_XP + 4            # 247: 8 f32 cells = 16 packed fp16 X' values
C_PM = C_XP16 + 8            # 255: [-1.0, 1.0] pair for the comp1 swap
C_Y4 = C_PM + 2              # 257: xyz[:,1] on partitions 0..3
C_L16 = C_Y4 + 1             # 258: fp16 [ [1;tilt] | wmat ] (9x42 halves)
C_OFF16 = C_L16 + 21         # 279: fp16 off row (80 halves)
C_ONES16 = C_OFF16 + 40      # 319: fp16 ones[4] for the off rank-1
PACK_COLS = C_ONES16 + 2     # 321

KEEP = np.r_[0:40, 58:160]

AFT = mybir.ActivationFunctionType
F32 = mybir.dt.float32
F16 = mybir.dt.float16


def _build_wmat() -> np.ndarray:
    views = np.arange(N_VIEWS)
    idx1 = np.searchsorted(CHECK_LIST, views, side="right") - 1
    idx2 = np.minimum(idx1 + 1, len(CHECK_LIST) - 1)
    denom = (CHECK_LIST[idx2] - CHECK_LIST[idx1]).astype(np.float32)
    denom[denom == 0] = 1.0
    frac = (views - CHECK_LIST[idx1]).astype(np.float32) / denom
    w = np.zeros((N_VIEWS, N_TILT), dtype=np.float64)
    c = np.zeros(N_VIEWS, dtype=np.float64)
    for v in range(N_VIEWS):
        if v == 14:
            c[v] = -15.0
        else:
            w[v, idx1[v]] += 1.0 - float(frac[v])
            w[v, idx2[v]] += float(frac[v])
    wmat = np.concatenate([c[None, :], w.T], axis=0) * DEG2RAD
    return np.ascontiguousarray(wmat, dtype=np.float32)


_WMAT = _build_wmat()
_NC_CACHE: list = []

# Experiment gates (env): BASS_MAX_SEM=<n> adds --max-sem-num to walrus,
# BASS_KEEP_END=1 keeps the TileContext end-block barriers.
_MAX_SEM = os.environ.get("BASS_MAX_SEM")
_STRIP_END = os.environ.get("BASS_KEEP_END") != "1"
_TAG = f"v17_ms{_MAX_SEM or 'x'}se{int(_STRIP_END)}"

_ORIG_WALRUS_ARGS = None


def _install_walrus_patch():
    """Append --max-sem-num to the walrus invocation for our compiles."""
    global _ORIG_WALRUS_ARGS
    if _MAX_SEM is None or _ORIG_WALRUS_ARGS is not None:
        return
    import concourse.bass_utils as bu

    _ORIG_WALRUS_ARGS = bu.get_walrus_args

    def _patched(arch, tmpdir, *, dve_root=None):
        return [f"--max-sem-num={_MAX_SEM}"] + _ORIG_WALRUS_ARGS(
            arch, tmpdir, dve_root=dve_root)

    bu.get_walrus_args = _patched


def _chain(insts):
    for a, b in zip(insts, insts[1:]):
        add_dep_helper(b.ins, a.ins, sync=False, reason="pin engine order")


_ENG_SEM_PREFIX = {
    "Pool": "Pool", "Activation": "Activation", "PE": "PE", "DVE": "DVE",
}

# earliest-firing sems first so carrier stalls overlap earlier waits;
# DVE (the longest product chain) is usually the true gate -> last.
_WAIT_ORDER = {"DMAHW": 0, "Sync": 1, "SP": 1, "Pool": 2, "Activation": 3,
               "PE": 4, "DVE": 5}


def _strip_self_waits(nc) -> None:
    """Drop sem waits on the instruction's own engine: the queue is
    in-order, so they are redundant and would cost a carrier each."""
    for fn in nc.m.functions:
        for blk in fn.blocks:
            for inst in blk.instructions:
                si = inst.sync_info
                if si is None or not si.on_wait:
                    continue
                pref = _ENG_SEM_PREFIX.get(getattr(inst.engine, "value",
                                                   str(inst.engine)))
                if pref is None:
                    continue
                kept = [w for w in si.on_wait
                        if not str(getattr(w, "ant_name", "")).startswith(
                            pref + "_")]
                if len(kept) != len(si.on_wait) and kept:
                    si.on_wait = kept


def _legalize_multiwait(nc) -> None:
    """walrus fits one sem-wait per instruction; hoist extras onto
    single-wait EventSemaphore carriers (earliest-firing first)."""

    def prio(w):
        nm = str(getattr(w, "ant_name", ""))
        base = nm.split("_")[0]
        for k, v in _WAIT_ORDER.items():
            if base.startswith(k):
                return v
        return 9

    for fn in nc.m.functions:
        for blk in fn.blocks:
            il = blk.instructions
            i = 0
            while i < len(il):
                inst = il[i]
                si = inst.sync_info
                if si is not None and si.on_wait is not None and len(si.on_wait) > 1:
                    waits = sorted(si.on_wait, key=prio)
                    extras, keep = waits[:-1], waits[-1]
                    for j, w in enumerate(extras):
                        ev = mybir.InstEventSemaphore(
                            name=f"{inst.name}_wsplit{j}")
                        ev.engine = inst.engine
                        try:
                            ev.sync_info.on_wait = [w]
                        except Exception:
                            ev.sync_info = mybir.SyncInfo(on_wait=[w],
                                                          on_update=[])
                        il.insert(i, ev)
                        i += 1
                    si.on_wait = [keep]
                i += 1


def _strip_preamble(nc) -> None:
    """Drop the const-AP memsets and the init all-engine barrier (nothing
    uses the const APs; all cross-engine ordering is via tile sems)."""
    il = nc.m.functions[0].blocks[0].instructions
    keep = []
    for inst in il:
        nm = type(inst).__name__
        if nm == "InstMemset" and "const-" in str(inst.outs[0]):
            continue
        if nm in ("InstDrain", "InstEventSemaphore", "InstRegisterMove"):
            continue
        keep.append(inst)
    il[:] = keep


def _strip_endblock(nc) -> None:
    """Drop the TileContext exit barriers/sem clears (the Sync queue's
    blocking out-DMA already orders completion; the runtime wrapper
    re-clears every semaphore anyway), then the now-empty end block and
    the per-engine branches into it: Sync's branch ran AFTER the out-DMA
    and extended the measured window by ~120ns."""
    for fn in nc.m.functions:
        for blk in fn.blocks:
            if blk.name.endswith("_end"):
                blk.instructions[:] = [
                    i for i in blk.instructions
                    if type(i).__name__ not in
                    ("InstEventSemaphore", "InstDrain", "InstISA")
                ]
    for fn in nc.m.functions:
        empty_ends = {b.name for b in fn.blocks
                      if b.name.endswith("_end") and not b.instructions}
        if not empty_ends:
            continue
        for blk in fn.blocks:
            blk.instructions[:] = [
                i for i in blk.instructions
                if not (type(i).__name__ == "InstUnconditionalBranch"
                        and str(getattr(i, "target", "")) in empty_ends)
            ]
        fn.blocks[:] = [b for b in fn.blocks if b.name not in empty_ends]


def _build_nc(postpasses: bool = True) -> bass.Bass:
    nc = bass.Bass("TRN2", target_bir_lowering=False, debug=False,
                   num_devices=N_CORES)

    pack_d = nc.dram_tensor(f"pack_{_TAG}", [N_TILT + 1, PACK_COLS], F32,
                            kind="ExternalInput")
    out_d = nc.dram_tensor("out", [4 * V, 2], F32, kind="ExternalOutput")

    with tile.TileContext(nc) as tc:
        with (
            tc.tile_pool(name="sb", bufs=1) as sb,
            tc.tile_pool(name="ps", bufs=1, space="PSUM") as ps,
        ):
            pk = sb.tile([N_TILT + 1, PACK_COLS], F32)
            tilt_ps = ps.tile([1, V], F32)
            tilt_ps2 = ps.tile([1, V], F32)
            crsr = sb.tile([1, W2], F32)   # [cr|sr] interleaved pairs
            ct = sb.tile([1, V], F32)
            st = sb.tile([1, V], F32)
            mcs = sb.tile([1, W2], F32)    # [mc|ms] interleaved pairs
            stage = sb.tile([1, 3 * W2], F16)  # [comp0 | comp1 | comp2]
            uv_ps = ps.tile([N_MARKERS, W2], F32)
            out_sb = sb.tile([N_MARKERS, W2], F32)

            halfpi_ap = pk[0:1, C_HPI:C_HPI + 1]
            zero_ap = pk[0:1, C_ZERO:C_ZERO + 1]
            rot_ap = pk[0:1, C_ROT:C_ROT + V]
            magi_ap = pk[0:1, C_MAGI:C_MAGI + W2]
            off_ap = pk[0:1, C_OFF:C_OFF + W2]

            # ---- single input DMA (free preamble) -----------------------
            d_in = nc.sync.dma_start(pk[:, :], pack_d.ap())

            # ---- PE (all fp16, 1-pass): tilt radians TWICE into two
            # separate PSUM banks (ACT PSUM reads to one bank serialize:
            # a_st waited for a_ct to retire), then the off rank-1 -------
            lw16 = pk[0:9, C_L16:C_L16 + 21].bitcast(F16)   # [9, 42]
            off16 = pk[0:1, C_OFF16:C_OFF16 + 40].bitcast(F16)  # [1, 80]
            ones16 = pk[0:1, C_ONES16:C_ONES16 + 2].bitcast(F16)  # [1, 4]
            mm1 = nc.tensor.matmul(tilt_ps[:, :], lw16[:, 0:1],
                                   lw16[:, 1:1 + V])
            mm1b = nc.tensor.matmul(tilt_ps2[:, :], lw16[:, 0:1],
                                    lw16[:, 1:1 + V])
            mm_off = nc.tensor.matmul(uv_ps[:, :], ones16, off16,
                                      start=True, stop=False)

            # ---- ACT: sr -> cr -> ct -> st ------------------------------
            # sr/cr write interleaved [cr,sr] pairs; ct = sin(pi/2 - t).
            a_sr = nc.scalar.activation(crsr[0:1, 1:W2:2], rot_ap, AFT.Sin,
                                        bias=zero_ap, scale=DEG2RAD)
            a_cr = nc.scalar.activation(crsr[0:1, 0:W2:2], rot_ap, AFT.Sin,
                                        bias=halfpi_ap, scale=DEG2RAD)
            a_ct = nc.scalar.activation(ct[:, :], tilt_ps[:, :], AFT.Sin,
                                        bias=halfpi_ap, scale=-1.0)
            a_st = nc.scalar.activation(st[:, :], tilt_ps2[:, :], AFT.Sin,
                                        bias=zero_ap)
            _chain([a_sr, a_cr, a_ct, a_st])

            # ---- DVE: mcs, then comp0/comp2 as single 3D-broadcast ops --
            s1, s2 = W2, 2 * W2
            mcs3 = mcs[0:1, 0:W2].rearrange("p (v r) -> p v r", v=V, r=2)
            sg0 = stage[0:1, 0:s1].rearrange("p (v r) -> p v r", v=V, r=2)
            sg2 = stage[0:1, s2:3 * W2].rearrange("p (v r) -> p v r", v=V,
                                                  r=2)
            ct_b = ct[0:1, 0:V].unsqueeze(2).broadcast_to([1, V, 2])
            st_b = st[0:1, 0:V].unsqueeze(2).broadcast_to([1, V, 2])
            v1 = nc.vector.tensor_mul(mcs[:, :], magi_ap, crsr[:, :])
            v2 = nc.vector.tensor_mul(sg0, mcs3, ct_b)
            # comp1 = [-ms, mc] from the mcs pairs, scheduled BEFORE the
            # st products: it only needs mcs, and running it last made it
            # gate the stop matmul (DVE is serial; Pool is ~4x slower)
            g1 = nc.vector.tensor_scalar_mul(stage[0:1, s1:s2:2],
                                             mcs[0:1, 1:W2:2], -1.0)
            g2 = nc.vector.tensor_copy(stage[0:1, s1 + 1:s2:2],
                                       mcs[0:1, 0:W2:2])
            v3 = nc.vector.tensor_mul(sg2, mcs3, st_b)
            _chain([v1, v2, g1, g2, v3])

            # ---- PE: accumulate the three component rank-1 terms --------
            xp16 = pk[0:1, C_XP16:C_XP16 + 8].bitcast(F16)
            mm_s0 = nc.tensor.matmul(uv_ps[:, :], xp16[0:1, 0:4],
                                     stage[0:1, 0:s1],
                                     start=False, stop=False)
            mm_s1 = nc.tensor.matmul(uv_ps[:, :], xp16[0:1, 4:8],
                                     stage[0:1, s1:s2],
                                     start=False, stop=False)
            mm_s2 = nc.tensor.matmul(uv_ps[:, :], xp16[0:1, 8:12],
                                     stage[0:1, s2:3 * W2],
                                     start=False, stop=True)
            _chain([mm1, mm1b, mm_off, mm_s0, mm_s1, mm_s2])

            # ---- PSUM -> SBUF -> out DMA --------------------------------
            v_fin = nc.vector.tensor_copy(out_sb[:, :], uv_ps[:, :])
            _chain([v3, v_fin])
            d_out = nc.sync.dma_start(out_d.ap(), out_sb[:, :],
                                      single_packet=True)
            _chain([d_in, d_out])

    if postpasses:
        _strip_self_waits(nc)
        _legalize_multiwait(nc)
        _strip_preamble(nc)
        if _STRIP_END:
            _strip_endblock(nc)
    return nc


def _make_in_map(inputs: dict) -> dict:
    tilt = np.ascontiguousarray(inputs["tilt_angles"], dtype=np.float32)
    xyz = np.ascontiguousarray(inputs["xyz"], dtype=np.float32)
    mag_eff = np.ascontiguousarray(inputs["mag"], np.float32).copy()
    mag_eff[0] = 1.0
    off_eff = np.ascontiguousarray(inputs["offset"], np.float32).copy()
    off_eff[0] = 0.0
    pack = np.zeros((N_TILT + 1, PACK_COLS), np.float32)
    pack[0, 0] = 1.0
    pack[1:, 0] = tilt
    pack[:, 1:1 + N_VIEWS] = _WMAT
    pack[0, C_HPI] = HALF_PI
    pack[0, C_ROT:C_MAGI] = np.ascontiguousarray(inputs["rot_angles"],
                                                 np.float32)
    pack[0, C_MAGI:C_OFF] = np.repeat(mag_eff, 2)
    pack[0, C_OFF:C_XP] = off_eff.reshape(-1)
    # ones row (fp32) for the off rank-1; X' columns packed as fp16
    pack[0, C_XP:C_XP + 4] = 1.0
    h = np.zeros(16, np.float16)
    h[0:4] = xyz[:, 0]
    h[4:8] = xyz[:, 1]
    h[8:12] = xyz[:, 2]
    pack[0, C_XP16:C_XP16 + 8] = h.view(np.float32)
    pack[0, C_PM] = -1.0
    pack[0, C_PM + 1] = 1.0
    pack[0:4, C_Y4] = xyz[:, 1]
    lw = np.zeros((9, 42), np.float16)
    lw[0, 0] = 1.0
    lw[1:, 0] = tilt
    lw[:, 1:41] = _WMAT
    pack[:, C_L16:C_L16 + 21] = lw.view(np.float32)
    o16 = np.zeros(80, np.float16)
    o16[:] = off_eff.reshape(-1)
    pack[0, C_OFF16:C_OFF16 + 40] = o16.view(np.float32)
    e16 = np.ones(4, np.float16)
    pack[0, C_ONES16:C_ONES16 + 2] = e16.view(np.float32)
    return {f"pack_{_TAG}": pack}


def kernel(**inputs: np.ndarray) -> np.ndarray:
    _install_walrus_patch()
    if not _NC_CACHE:
        _NC_CACHE.append(_build_nc())
    nc = _NC_CACHE[0]

    in_map = _make_in_map(inputs)
    core_ids = list(range(N_CORES))
    res = run_bass_kernel_spmd(nc, [in_map] * N_CORES, core_ids)
    full = np.asarray(res.results[0]["out"], dtype=np.float32)
    return np.ascontiguousarray(full[KEEP])


# revision 37
# speedup vs baseline: 1.1923x; 1.0012x over previous
"""Final kernel (HW: ~10.0us at fast clocks, rel err 6.86e-4 vs 2e-2 gate).

Measurement window = [first compute-class instruction -> absolute end of
the execution, including the runtime's fixed ~7.4us semaphore-sweep
teardown that follows every NEFF execution].  A trivial copy kernel
measures ~8.3us on this runtime, so only the compute chain is winnable.

vs the 12.7us baseline:
- TileContext end-block barriers stripped (-1.5us): the blocking out-DMA
  already orders the Sync queue, and the runtime wrapper re-clears every
  semaphore anyway.
- 4 activations instead of 5: ct = sin(pi/2 - t) directly (cos is even,
  so no |t| needed; max arg 2.35 rad is within the Sin PWP range); tilt
  is computed TWICE into two separate PSUM tiles (fp16 1-pass matmuls)
  because ACT PSUM reads to one bank serialize: with a private bank the
  4th activation pipelines at the ~220ns cadence instead of stalling.
- cr/sr written interleaved by the two rot ACTs, so one DVE multiply
  against a host-packed interleaved mag vector forms [mc,ms] pairs, and
  comp0/comp2 are single 3D-broadcast DVE ops (ct/st read stride-0).
- comp1 = [-ms, mc] via two strided DVE ops (Pool is ~4x slower per tiny
  op), scheduled BEFORE the st-gated products so it never gates the PE.
- projection matmuls in fp16 (lhsT = X' packed as fp16 bitcast): 1-pass
  on the PE instead of fp32's 2-pass; issued in product-readiness order
  with the stop flag on the last gate.
- post-passes: drop same-engine sem waits, hoist multi-waits onto
  single-wait carriers (walrus limit), strip framework preamble movs,
  drop the per-engine branches to the empty end block (Sync's ran after
  the out-DMA and extended the measured window)."""

import os

import numpy as np

import concourse.bass as bass
import concourse.mybir as mybir
from concourse import tile
from concourse.tile_rust import add_dep_helper
from concourse.bass_utils import run_bass_kernel_spmd

N_VIEWS = 40
N_MARKERS = 4
N_TILT = 8
PI = float(np.pi)
DEG2RAD = PI / 180.0
HALF_PI = PI / 2.0
CHECK_LIST = np.array([0, 3, 8, 20, 26, 32, 36, 39])
N_CORES = 8

V = N_VIEWS
W2 = 2 * V

# pack layout ([9, 259], row 0 unless noted):
#   col 0              [1; tilt_angles]            (partitions 0..8)
#   cols 1..41         wmat (radians)              (partitions 0..8)
#   col 41             pi/2
#   col 42             0.0
#   cols 43..83        rot_angles (degrees)
#   cols 83..163       mag_eff interleaved [m0,m0,m1,m1,...] (view-0 -> 1)
#   cols 163..243      off_eff flattened (view-0 zeroed)
#   cols 243..259      X'^T rows as [1,4] blocks per seg: [1s, x, y, z]
C_HPI = 1 + V                # 41
C_ZERO = C_HPI + 1           # 42
C_ROT = C_ZERO + 1           # 43
C_MAGI = C_ROT + V           # 83
C_OFF = C_MAGI + W2          # 163
C_XP = C_OFF + W2            # 243: fp32 ones row for the off rank-1
C_XP16 = C_XP + 4            # 247: 8 f32 cells = 16 packed fp16 X' values
C_PM = C_XP16 + 8            # 255: [-1.0, 1.0] pair for the comp1 swap
C_Y4 = C_PM + 2              # 257: xyz[:,1] on partitions 0..3
C_L16 = C_Y4 + 1             # 258: fp16 [ [1;tilt] | wmat ] (9x42 halves)
C_OFF16 = C_L16 + 21         # 279: fp16 off row (80 halves)
C_ONES16 = C_OFF16 + 40      # 319: fp16 ones[4] for the off rank-1
PACK_COLS = C_ONES16 + 2     # 321

KEEP = np.r_[0:40, 58:160]

AFT = mybir.ActivationFunctionType
F32 = mybir.dt.float32
F16 = mybir.dt.float16


def _build_wmat() -> np.ndarray:
    views = np.arange(N_VIEWS)
    idx1 = np.searchsorted(CHECK_LIST, views, side="right") - 1
    idx2 = np.minimum(idx1 + 1, len(CHECK_LIST) - 1)
    denom = (CHECK_LIST[idx2] - CHECK_LIST[idx1]).astype(np.float32)
    denom[denom == 0] = 1.0
    frac = (views - CHECK_LIST[idx1]).astype(np.float32) / denom
    w = np.zeros((N_VIEWS, N_TILT), dtype=np.float64)
    c = np.zeros(N_VIEWS, dtype=np.float64)
    for v in range(N_VIEWS):
        if v == 14:
            c[v] = -15.0
        else:
            w[v, idx1[v]] += 1.0 - float(frac[v])
            w[v, idx2[v]] += float(frac[v])
    wmat = np.concatenate([c[None, :], w.T], axis=0) * DEG2RAD
    return np.ascontiguousarray(wmat, dtype=np.float32)


_WMAT = _build_wmat()
_NC_CACHE: list = []

# Experiment gates (env): BASS_MAX_SEM=<n> adds --max-sem-num to walrus,
# BASS_KEEP_END=1 keeps the TileContext end-block barriers.
_MAX_SEM = os.environ.get("BASS_MAX_SEM")
_STRIP_END = os.environ.get("BASS_KEEP_END") != "1"
_TAG = f"v20_ms{_MAX_SEM or 'x'}se{int(_STRIP_END)}"

_ORIG_WALRUS_ARGS = None


def _install_walrus_patch():
    """Append --max-sem-num to the walrus invocation for our compiles."""
    global _ORIG_WALRUS_ARGS
    if _MAX_SEM is None or _ORIG_WALRUS_ARGS is not None:
        return
    import concourse.bass_utils as bu

    _ORIG_WALRUS_ARGS = bu.get_walrus_args

    def _patched(arch, tmpdir, *, dve_root=None):
        return [f"--max-sem-num={_MAX_SEM}"] + _ORIG_WALRUS_ARGS(
            arch, tmpdir, dve_root=dve_root)

    bu.get_walrus_args = _patched


def _chain(insts):
    for a, b in zip(insts, insts[1:]):
        add_dep_helper(b.ins, a.ins, sync=False, reason="pin engine order")


_ENG_SEM_PREFIX = {
    "Pool": "Pool", "Activation": "Activation", "PE": "PE", "DVE": "DVE",
}

# earliest-firing sems first so carrier stalls overlap earlier waits;
# DVE (the longest product chain) is usually the true gate -> last.
_WAIT_ORDER = {"DMAHW": 0, "Sync": 1, "SP": 1, "Pool": 2, "Activation": 3,
               "PE": 4, "DVE": 5}


def _strip_self_waits(nc) -> None:
    """Drop sem waits on the instruction's own engine: the queue is
    in-order, so they are redundant and would cost a carrier each."""
    for fn in nc.m.functions:
        for blk in fn.blocks:
            for inst in blk.instructions:
                si = inst.sync_info
                if si is None or not si.on_wait:
                    continue
                pref = _ENG_SEM_PREFIX.get(getattr(inst.engine, "value",
                                                   str(inst.engine)))
                if pref is None:
                    continue
                kept = [w for w in si.on_wait
                        if not str(getattr(w, "ant_name", "")).startswith(
                            pref + "_")]
                if len(kept) != len(si.on_wait) and kept:
                    si.on_wait = kept


def _legalize_multiwait(nc) -> None:
    """walrus fits one sem-wait per instruction; hoist extras onto
    single-wait EventSemaphore carriers (earliest-firing first)."""

    def prio(w):
        nm = str(getattr(w, "ant_name", ""))
        base = nm.split("_")[0]
        for k, v in _WAIT_ORDER.items():
            if base.startswith(k):
                return v
        return 9

    for fn in nc.m.functions:
        for blk in fn.blocks:
            il = blk.instructions
            i = 0
            while i < len(il):
                inst = il[i]
                si = inst.sync_info
                if si is not None and si.on_wait is not None and len(si.on_wait) > 1:
                    waits = sorted(si.on_wait, key=prio)
                    extras, keep = waits[:-1], waits[-1]
                    for j, w in enumerate(extras):
                        ev = mybir.InstEventSemaphore(
                            name=f"{inst.name}_wsplit{j}")
                        ev.engine = inst.engine
                        try:
                            ev.sync_info.on_wait = [w]
                        except Exception:
                            ev.sync_info = mybir.SyncInfo(on_wait=[w],
                                                          on_update=[])
                        il.insert(i, ev)
                        i += 1
                    si.on_wait = [keep]
                i += 1


def _strip_preamble(nc) -> None:
    """Drop the const-AP memsets and the init all-engine barrier (nothing
    uses the const APs; all cross-engine ordering is via tile sems)."""
    il = nc.m.functions[0].blocks[0].instructions
    keep = []
    for inst in il:
        nm = type(inst).__name__
        if nm == "InstMemset" and "const-" in str(inst.outs[0]):
            continue
        if nm in ("InstDrain", "InstEventSemaphore", "InstRegisterMove"):
            continue
        keep.append(inst)
    il[:] = keep


def _strip_endblock(nc) -> None:
    """Drop the TileContext exit barriers/sem clears (the Sync queue's
    blocking out-DMA already orders completion; the runtime wrapper
    re-clears every semaphore anyway), then the now-empty end block and
    the per-engine branches into it: Sync's branch ran AFTER the out-DMA
    and extended the measured window by ~120ns."""
    for fn in nc.m.functions:
        for blk in fn.blocks:
            if blk.name.endswith("_end"):
                blk.instructions[:] = [
                    i for i in blk.instructions
                    if type(i).__name__ not in
                    ("InstEventSemaphore", "InstDrain", "InstISA")
                ]
    for fn in nc.m.functions:
        empty_ends = {b.name for b in fn.blocks
                      if b.name.endswith("_end") and not b.instructions}
        if not empty_ends:
            continue
        for blk in fn.blocks:
            blk.instructions[:] = [
                i for i in blk.instructions
                if not (type(i).__name__ == "InstUnconditionalBranch"
                        and str(getattr(i, "target", "")) in empty_ends)
            ]
        fn.blocks[:] = [b for b in fn.blocks if b.name not in empty_ends]


def _build_nc(postpasses: bool = True) -> bass.Bass:
    nc = bass.Bass("TRN2", target_bir_lowering=False, debug=False,
                   num_devices=N_CORES)

    pack_d = nc.dram_tensor(f"pack_{_TAG}", [N_TILT + 1, PACK_COLS], F32,
                            kind="ExternalInput")
    out_d = nc.dram_tensor("out", [4 * V, 2], F32, kind="ExternalOutput")

    with tile.TileContext(nc) as tc:
        with (
            tc.tile_pool(name="sb", bufs=1) as sb,
            tc.tile_pool(name="ps", bufs=1, space="PSUM") as ps,
        ):
            pk = sb.tile([N_TILT + 1, PACK_COLS], F32)
            tilt_ps = ps.tile([1, V], F32)
            tilt_ps2 = ps.tile([1, V], F32)
            crsr = sb.tile([1, W2], F32)   # [cr|sr] interleaved pairs
            ct = sb.tile([1, V], F32)
            st = sb.tile([1, V], F32)
            mcs = sb.tile([1, W2], F32)    # [mc|ms] interleaved pairs
            stage = sb.tile([1, 3 * W2], F16)  # [comp0 | comp1 | comp2]
            uv_ps = ps.tile([N_MARKERS, W2], F32)
            out_sb = sb.tile([N_MARKERS, W2], F32)

            halfpi_ap = pk[0:1, C_HPI:C_HPI + 1]
            zero_ap = pk[0:1, C_ZERO:C_ZERO + 1]
            rot_ap = pk[0:1, C_ROT:C_ROT + V]
            magi_ap = pk[0:1, C_MAGI:C_MAGI + W2]
            off_ap = pk[0:1, C_OFF:C_OFF + W2]

            # ---- single input DMA (free preamble) -----------------------
            d_in = nc.sync.dma_start(pk[:, :], pack_d.ap())

            # ---- PE (all fp16, 1-pass): tilt radians TWICE into two
            # separate PSUM banks (ACT PSUM reads to one bank serialize:
            # a_st waited for a_ct to retire), then the off rank-1 -------
            lw16 = pk[0:9, C_L16:C_L16 + 21].bitcast(F16)   # [9, 42]
            off16 = pk[0:1, C_OFF16:C_OFF16 + 40].bitcast(F16)  # [1, 80]
            ones16 = pk[0:1, C_ONES16:C_ONES16 + 2].bitcast(F16)  # [1, 4]
            mm1 = nc.tensor.matmul(tilt_ps[:, :], lw16[:, 0:1],
                                   lw16[:, 1:1 + V])
            mm1b = nc.tensor.matmul(tilt_ps2[:, :], lw16[:, 0:1],
                                    lw16[:, 1:1 + V])
            mm_off = nc.tensor.matmul(uv_ps[:, :], ones16, off16,
                                      start=True, stop=False)

            # ---- ACT: sr -> cr -> ct -> st ------------------------------
            # sr/cr write interleaved [cr,sr] pairs; ct = sin(pi/2 - t).
            a_sr = nc.scalar.activation(crsr[0:1, 1:W2:2], rot_ap, AFT.Sin,
                                        bias=zero_ap, scale=DEG2RAD)
            a_cr = nc.scalar.activation(crsr[0:1, 0:W2:2], rot_ap, AFT.Sin,
                                        bias=halfpi_ap, scale=DEG2RAD)
            a_ct = nc.scalar.activation(ct[:, :], tilt_ps[:, :], AFT.Sin,
                                        bias=halfpi_ap, scale=-1.0)
            a_st = nc.scalar.activation(st[:, :], tilt_ps2[:, :], AFT.Sin,
                                        bias=zero_ap)
            _chain([a_sr, a_cr, a_ct, a_st])

            # ---- DVE: mcs, then comp0/comp2 as single 3D-broadcast ops --
            s1, s2 = W2, 2 * W2
            mcs3 = mcs[0:1, 0:W2].rearrange("p (v r) -> p v r", v=V, r=2)
            sg0 = stage[0:1, 0:s1].rearrange("p (v r) -> p v r", v=V, r=2)
            sg2 = stage[0:1, s2:3 * W2].rearrange("p (v r) -> p v r", v=V,
                                                  r=2)
            ct_b = ct[0:1, 0:V].unsqueeze(2).broadcast_to([1, V, 2])
            st_b = st[0:1, 0:V].unsqueeze(2).broadcast_to([1, V, 2])
            sg1 = stage[0:1, s1:s2].rearrange("p (v r) -> p v r", v=V,
                                              r=2)
            pm_b = pk[0:1, C_PM:C_PM + 2].unsqueeze(1).broadcast_to(
                [1, V, 2])
            v1 = nc.vector.tensor_mul(mcs[:, :], magi_ap, crsr[:, :])
            v2 = nc.vector.tensor_mul(sg0, mcs3, ct_b)
            # comp1 = [-ms, mc]: swapped mcs pairs times [-1, +1] in ONE
            # op (scheduled before the st product: it only needs mcs),
            # so seg2 issues one DVE cadence slot earlier
            g1 = nc.vector.tensor_mul(sg1, mcs3[:, :, ::-1], pm_b)
            v3 = nc.vector.tensor_mul(sg2, mcs3, st_b)
            _chain([v1, v2, g1, v3])

            # ---- PE: accumulate the three component rank-1 terms --------
            xp16 = pk[0:1, C_XP16:C_XP16 + 8].bitcast(F16)
            mm_s0 = nc.tensor.matmul(uv_ps[:, :], xp16[0:1, 0:4],
                                     stage[0:1, 0:s1],
                                     start=False, stop=False)
            mm_s1 = nc.tensor.matmul(uv_ps[:, :], xp16[0:1, 4:8],
                                     stage[0:1, s1:s2],
                                     start=False, stop=False)
            mm_s2 = nc.tensor.matmul(uv_ps[:, :], xp16[0:1, 8:12],
                                     stage[0:1, s2:3 * W2],
                                     start=False, stop=True)
            _chain([mm1, mm1b, mm_off, mm_s0, mm_s1, mm_s2])

            # ---- PSUM -> SBUF -> out DMA --------------------------------
            v_fin = nc.vector.tensor_copy(out_sb[:, :], uv_ps[:, :])
            _chain([v3, v_fin])
            d_out = nc.sync.dma_start(out_d.ap(), out_sb[:, :],
                                      single_packet=True)
            _chain([d_in, d_out])

    if postpasses:
        _strip_self_waits(nc)
        _legalize_multiwait(nc)
        _strip_preamble(nc)
        if _STRIP_END:
            _strip_endblock(nc)
    return nc


def _make_in_map(inputs: dict) -> dict:
    tilt = np.ascontiguousarray(inputs["tilt_angles"], dtype=np.float32)
    xyz = np.ascontiguousarray(inputs["xyz"], dtype=np.float32)
    mag_eff = np.ascontiguousarray(inputs["mag"], np.float32).copy()
    mag_eff[0] = 1.0
    off_eff = np.ascontiguousarray(inputs["offset"], np.float32).copy()
    off_eff[0] = 0.0
    pack = np.zeros((N_TILT + 1, PACK_COLS), np.float32)
    pack[0, 0] = 1.0
    pack[1:, 0] = tilt
    pack[:, 1:1 + N_VIEWS] = _WMAT
    pack[0, C_HPI] = HALF_PI
    pack[0, C_ROT:C_MAGI] = np.ascontiguousarray(inputs["rot_angles"],
                                                 np.float32)
    pack[0, C_MAGI:C_OFF] = np.repeat(mag_eff, 2)
    pack[0, C_OFF:C_XP] = off_eff.reshape(-1)
    # ones row (fp32) for the off rank-1; X' columns packed as fp16
    pack[0, C_XP:C_XP + 4] = 1.0
    h = np.zeros(16, np.float16)
    h[0:4] = xyz[:, 0]
    h[4:8] = xyz[:, 1]
    h[8:12] = xyz[:, 2]
    pack[0, C_XP16:C_XP16 + 8] = h.view(np.float32)
    pack[0, C_PM] = -1.0
    pack[0, C_PM + 1] = 1.0
    pack[0:4, C_Y4] = xyz[:, 1]
    lw = np.zeros((9, 42), np.float16)
    lw[0, 0] = 1.0
    lw[1:, 0] = tilt
    lw[:, 1:41] = _WMAT
    pack[:, C_L16:C_L16 + 21] = lw.view(np.float32)
    o16 = np.zeros(80, np.float16)
    o16[:] = off_eff.reshape(-1)
    pack[0, C_OFF16:C_OFF16 + 40] = o16.view(np.float32)
    e16 = np.ones(4, np.float16)
    pack[0, C_ONES16:C_ONES16 + 2] = e16.view(np.float32)
    return {f"pack_{_TAG}": pack}


def kernel(**inputs: np.ndarray) -> np.ndarray:
    _install_walrus_patch()
    if not _NC_CACHE:
        _NC_CACHE.append(_build_nc())
    nc = _NC_CACHE[0]

    in_map = _make_in_map(inputs)
    core_ids = list(range(N_CORES))
    res = run_bass_kernel_spmd(nc, [in_map] * N_CORES, core_ids)
    full = np.asarray(res.results[0]["out"], dtype=np.float32)
    return np.ascontiguousarray(full[KEEP])
